# revision 22
# baseline (speedup 1.0000x reference)
"""Causal self-attention (B=4, T=2048, C=1024, H=16) on 8 trn2 NeuronCores.

Sharding: head-pair parallel. Core c owns heads {2c, 2c+1} for all 4 batches.
 - host: x pre-transposed to xT [C, B*T] bf16; W_qkv pre-sliced per core into
   wq/wk/wv [C, 128] bf16 (softmax scale folded into wq), W_proj bf16,
   f32 biases, and a packed bf16 constants tensor (identity, causal tril
   mask, ones) so the kernel needs no GpSimd compute ucode.
 - software pipeline: the attention tile loop of batch b doubles as the
   scheduler for everything else. qkv projections of batch b+1, the V
   transposes of b+1, and the out-projection of b-1 are emitted as small
   self-contained "filler units" between attention tiles, so the PE stays
   busy instead of pacing on ScalarE's exp. x tiles are prefetched two
   batches ahead ([128, 2048] per contraction chunk, one DMA each).
 - attention in S^T orientation: S^T[tk, tq] = kT.T@qT bf16 tiles
   [128, 512], diagonal tiles first. P^T = exp(S^T) on ScalarE (PSUM->SBUF
   bf16); the 128x128 causal wedge of diagonal tiles is zeroed on DVE by
   multiplying with a host-shipped 0/1 tril mask (exp of unmasked future
   logits is finite so this is safe). O-matmul lhsT = [v_h | ones] (M=65)
   gives O^T and the softmax denominator in one pass. Normalize: only the
   PSUM->SBUF copy happens at chunk end; reciprocal + ones-broadcast matmul
   + multiply + stores are deferred into the next chunk's tile loop.
 - per-batch AllToAll (512 KB/rank bf16) reshards O^T head->token shards.
   SBUF->DRAM stores go through the (otherwise idle) GpSimd DGE queue so
   they never head-of-line block the sync DGE queue that feeds loads and
   the collective. Column-parallel out-projection, bf16 out^T, host
   upcasts and reassembles.
"""
import numpy as np
import ml_dtypes
import concourse.bacc as bacc
import concourse.mybir as mybir
import concourse.tile as tile
from concourse.bass_utils import run_bass_kernel_spmd

F32 = mybir.dt.float32
BF16 = mybir.dt.bfloat16
Exp = mybir.ActivationFunctionType.Exp

NCORES = 8
B, T, C, H = 4, 2048, 1024, 16
HD = C // H          # 64
HL = H // NCORES     # 2 heads per core
D2 = HL * HD         # 128 rows of local head-pair dims
TB = T               # tokens per batch
NKC = C // 128       # 8 contraction chunks
NCH = TB // 512      # 4 tq chunks per batch
NTK = TB // 128      # 16 tk tiles per batch
PIECE = TB // NCORES  # 256 tokens per (batch, core) piece after AllToAll

# packed constants layout (bf16, [128, 480]):
#   cols 0:128  identity; 128:384 tril mask x2 heads; 384:416 ones (vh);
#   row 0 cols 416:480 ones (rb lhsT)
CN_ID = 0
CN_MASK = 128
CN_ONES = 384
CN_ONESR = 416
CN_W = 480

_CACHE = {}


def _build(sim=False):
    nc = bacc.Bacc("TRN2", target_bir_lowering=False, debug=False,
                   num_devices=1 if sim else NCORES)
    xt = nc.dram_tensor("xt", [C, B * T], BF16, kind="ExternalInput").ap()
    wq = nc.dram_tensor("wq", [C, D2], BF16, kind="ExternalInput").ap()
    wk = nc.dram_tensor("wk", [C, D2], BF16, kind="ExternalInput").ap()
    wv = nc.dram_tensor("wv", [C, D2], BF16, kind="ExternalInput").ap()
    wp = nc.dram_tensor("wp", [C, C], BF16, kind="ExternalInput").ap()
    cn = nc.dram_tensor("cn", [128, CN_W], BF16, kind="ExternalInput").ap()
    bqkv = nc.dram_tensor("bqkv", [D2, 3], F32, kind="ExternalInput").ap()
    bp = nc.dram_tensor("bp", [128, NKC], F32, kind="ExternalInput").ap()
    outp = nc.dram_tensor("outp", [C, B * PIECE], BF16,
                          kind="ExternalOutput").ap()

    inb = [nc.dram_tensor(f"inb{b}", [NCORES, D2, PIECE], BF16)
           for b in range(B)]
    outb = [nc.dram_tensor(f"outb{b}", [NCORES, D2, PIECE], BF16)
            for b in range(B)]

    with tile.TileContext(nc) as tc:
        with (
            tc.tile_pool(name="const", bufs=1) as cpool,
            tc.tile_pool(name="w", bufs=1) as wpool,
            tc.tile_pool(name="xt", bufs=16) as xpool,
            tc.tile_pool(name="qk", bufs=2) as qkpool,
            tc.tile_pool(name="vstg", bufs=1) as vstgpool,
            tc.tile_pool(name="vh", bufs=2) as vhpool,
            tc.tile_pool(name="pt", bufs=5) as ptpool,
            tc.tile_pool(name="small", bufs=3) as smallpool,
            tc.tile_pool(name="ofin", bufs=3) as ofinpool,
            tc.tile_pool(name="proj", bufs=3) as projpool,
            tc.tile_pool(name="otp", bufs=3) as otpool,
            tc.tile_pool(name="mm", bufs=1, space="PSUM") as mmps,
            tc.tile_pool(name="s", bufs=2, space="PSUM") as sps,
            tc.tile_pool(name="o", bufs=1, space="PSUM") as ops,
        ):
            # ---- constants / weights, ordered so batch-0 qkv starts asap
            wq_sb = wpool.tile([128, NKC, D2], BF16)
            wk_sb = wpool.tile([128, NKC, D2], BF16)
            wv_sb = wpool.tile([128, NKC, D2], BF16)
            nc.sync.dma_start(
                wq_sb[:], wq.rearrange("(kc p) m -> p kc m", p=128))

            xts = {}

            def emit_xt_loads(b, striped=False):
                xts[b] = [xpool.tile([128, TB], BF16, name="xtile")
                          for _ in range(NKC)]
                if striped:
                    # two half-tile DMAs per chunk so batch-0 compute can
                    # start after the first 8 arrive; wk/wv slot in between
                    for half in range(2):
                        for kc in range(NKC):
                            nc.sync.dma_start(
                                xts[b][kc][:, 1024 * half:1024 * (half + 1)],
                                xt[128 * kc:128 * (kc + 1),
                                   b * TB + 1024 * half:
                                   b * TB + 1024 * (half + 1)])
                        if half == 0:
                            for t, d in ((wk_sb, wk), (wv_sb, wv)):
                                nc.sync.dma_start(
                                    t[:],
                                    d.rearrange("(kc p) m -> p kc m", p=128))
                else:
                    for kc in range(NKC):
                        nc.sync.dma_start(
                            xts[b][kc][:],
                            xt[128 * kc:128 * (kc + 1), b * TB:(b + 1) * TB])

            bqkv_sb = cpool.tile([D2, 3], F32)
            nc.sync.dma_start(bqkv_sb[:], bqkv)
            cn_sb = cpool.tile([128, CN_W], BF16)
            nc.sync.dma_start(cn_sb[:], cn)
            emit_xt_loads(0, striped=True)
            identb = cn_sb[:, CN_ID:CN_ID + 128]
            maskb = cn_sb[:, CN_MASK:CN_MASK + 256].rearrange(
                "p (g c) -> p g c", g=HL)
            onesb = cn_sb[:, CN_ONES:CN_ONES + 32]
            onesr = cn_sb[0:1, CN_ONESR:CN_ONESR + 64]

            emit_xt_loads(1)
            wp_sb = wpool.tile([128, NKC, C], BF16)
            for g in range(4):
                nc.sync.dma_start(
                    wp_sb[:, 2 * g:2 * (g + 1), :],
                    wp[256 * g:256 * (g + 1), :].rearrange(
                        "(two p) m -> p two m", p=128))
            bp_sb = cpool.tile([128, NKC], F32)
            nc.sync.dma_start(bp_sb[:], bp)

            qT = {}
            kT = {}
            vh = {}
            vT = {}

            def qkv_units(b):
                """12 qkv matmul groups + 4 v-transpose groups for batch b."""
                qT[b] = qkpool.tile([D2, TB], BF16, tag="qT", name="qT")
                kT[b] = qkpool.tile([D2, TB], BF16, tag="kT", name="kT")
                vT[b] = vstgpool.tile([D2, TB], BF16, name="vT")
                units = []

                def qkv_group(n, col, w_sb, dst, b=b):
                    def emit():
                        ps = mmps.tile([128, 512], F32, tag="qps",
                                       name="ps")
                        for kc in range(NKC):
                            nc.tensor.matmul(
                                ps[:], w_sb[:, kc, :],
                                xts[b][kc][:, 512 * n:512 * (n + 1)],
                                start=(kc == 0), stop=(kc == NKC - 1))
                        with nc.allow_low_precision(reason="bf16 qkv"):
                            nc.vector.tensor_scalar_add(
                                dst[:, 512 * n:512 * (n + 1)], ps[:],
                                bqkv_sb[:, col:col + 1])
                        if n == NCH - 1 and col == 2:
                            del xts[b]
                    return emit

                def vt_group(tk0, b=b):
                    def emit():
                        # layout: vh[:, h*(NTK*65) + tk*65 + (0:64)] = v tile
                        if tk0 == 0:
                            vh[b] = vhpool.tile([128, HL * NTK * 65], BF16,
                                                tag="vh", name="vh")
                            with nc.allow_low_precision(reason="ones"):
                                nc.vector.tensor_copy(vh[b][:, 64::65],
                                                      onesb[:])
                        vt_ps = mmps.tile([128, 512], BF16, tag="vtps")
                        for i in range(4):
                            nc.tensor.transpose(
                                vt_ps[:, 128 * i:128 * (i + 1)],
                                vT[b][:, 128 * (tk0 + i):
                                      128 * (tk0 + i + 1)],
                                identb)
                        with nc.allow_low_precision(reason="bf16 v"):
                            nc.vector.tensor_copy(
                                vh[b][:].rearrange(
                                    "p (h tk c) -> p h tk c",
                                    h=HL, tk=NTK)[:, :, tk0:tk0 + 4, 0:64],
                                vt_ps[:].rearrange(
                                    "p (i h c) -> p h i c", i=4, h=HL))
                    return emit

                for n in range(NCH):
                    for col, (w_sb, dst) in enumerate(
                            ((wq_sb, qT[b]), (wk_sb, kT[b]),
                             (wv_sb, vT[b]))):
                        units.append(qkv_group(n, col, w_sb, dst))
                    units.append(vt_group(4 * n))
                return units

            ot = {}

            def emit_ot_load(b, split=False):
                ot[b] = otpool.tile([128, NCORES, PIECE], BF16, name="ot")
                if split:
                    for g in range(4):
                        nc.sync.dma_start(
                            ot[b][:, 2 * g:2 * (g + 1), :],
                            outb[b].ap()[2 * g:2 * (g + 1)].rearrange(
                                "s p u -> p s u"))
                else:
                    nc.sync.dma_start(
                        ot[b][:], outb[b].ap().rearrange("s p u -> p s u"))

            def outproj_units(b, store_eng=None, alt=False):
                """4 out-projection column-pair groups for batch b."""
                units = []

                def op_group(mc2, b=b):
                    def emit():
                        osb = projpool.tile([128, 2, PIECE], BF16,
                                            tag="osb", name="osb")
                        for sub in range(2):
                            mcol = 2 * mc2 + sub
                            # at the tail, alternate PSUM tags so groups
                            # double-buffer (keeps the PE p-state warm)
                            if alt and sub == 1:
                                pp = sps.tile([128, 1024], F32, tag="s_ps",
                                              name="pp")
                            else:
                                pp = mmps.tile([128, 512], F32, tag="qps",
                                               name="pp")
                            for s8 in range(NCORES):
                                nc.tensor.matmul(
                                    pp[:, 0:PIECE],
                                    wp_sb[:, s8,
                                          128 * mcol:128 * (mcol + 1)],
                                    ot[b][:, s8, :], start=(s8 == 0),
                                    stop=(s8 == NCORES - 1))
                            with nc.allow_low_precision(reason="bf16 out"):
                                nc.vector.tensor_scalar_add(
                                    osb[:, sub, :], pp[:, 0:PIECE],
                                    bp_sb[:, mcol:mcol + 1])
                            if store_eng is not None:
                                store_eng.dma_start(
                                    outp[256 * mc2 + 128 * sub:
                                         256 * mc2 + 128 * (sub + 1),
                                         PIECE * b:PIECE * (b + 1)],
                                    osb[:, sub, :])
                        if store_eng is None:
                            nc.gpsimd.dma_start(
                                outp[256 * mc2:256 * (mc2 + 1),
                                     PIECE * b:PIECE * (b + 1)].rearrange(
                                         "(two p) u -> p two u", p=128),
                                osb[:])
                    return emit

                for mc2 in range(NKC // 2):
                    units.append(op_group(mc2))
                return units

            def interleave(a, bl):
                """a with elements of bl spread through (a leads)."""
                if not bl:
                    return list(a)
                out = []
                step = max(1, len(a) // (len(bl) + 1))
                bi = 0
                for i, u in enumerate(a):
                    out.append(u)
                    if bi < len(bl) and (i + 1) % step == 0 and i >= 2:
                        out.append(bl[bi])
                        bi += 1
                out.extend(bl[bi:])
                return out

            pending = []
            final_tail = []

            def flush_pending():
                while pending:
                    pending.pop(0)()

            def emit_attn(b, units, tail_fn=None, bias=0):
                if b + 2 < B:
                    emit_xt_loads(b + 2)
                n_tiles = sum(4 * j + 4 for j in range(NCH))
                done = 0
                emitted = 0
                for j in range(NCH):
                    o_ps = ops.tile([65, 1024], F32, tag="o")
                    # full tiles first (no mask dependency) so the chunk
                    # opener never waits on the o_ps WAR or DVE masks; the
                    # first tile always spans the full width (m=0 when j=0)
                    seq = list(range(4 * j)) + list(range(4 * j, 4 * j + 4))
                    for idx, tk in enumerate(seq):
                        m = tk - 4 * j
                        z = 128 * m if m >= 0 else 0
                        s_ps = sps.tile([128, 1024], F32, tag="s_ps",
                                        name="s_ps")
                        for h in range(HL):
                            nc.tensor.matmul(
                                s_ps[:, 512 * h + z:512 * (h + 1)],
                                kT[b][64 * h:64 * (h + 1),
                                      128 * tk:128 * (tk + 1)],
                                qT[b][64 * h:64 * (h + 1),
                                      512 * j + z:512 * (j + 1)],
                                start=True, stop=True)
                        pt = ptpool.tile([128, 1024], BF16, tag="pt")
                        if z:
                            exp_src = s_ps[:].rearrange(
                                "p (g c) -> p g c", g=2)[:, :, z:]
                            exp_dst = pt[:].rearrange(
                                "p (g c) -> p g c", g=2)[:, :, z:]
                            nc.scalar.activation(exp_dst, exp_src, Exp)
                        else:
                            nc.scalar.activation(pt[:], s_ps[:], Exp)
                        if m >= 0:
                            # zero the causal wedge: keep where tq_l >= tk_l
                            ptv = pt[:].rearrange(
                                "p (g c) -> p g c", g=2)[:, :, z:z + 128]
                            with nc.allow_low_precision(reason="mask"):
                                nc.vector.tensor_mul(ptv, ptv, maskb)
                        for h in range(HL):
                            nc.tensor.matmul(
                                o_ps[0:65, 512 * h + z:512 * (h + 1)],
                                vh[b][:, (h * NTK + tk) * 65:
                                      (h * NTK + tk + 1) * 65],
                                pt[:, 512 * h + z:512 * (h + 1)],
                                start=(idx == 0), stop=(idx == len(seq) - 1))
                        done += 1
                        if idx == 2:
                            flush_pending()
                            if j == 0 and tail_fn is not None:
                                tail_fn()
                                tail_fn = None
                        # pace filler so it finishes ~4 tiles before the end
                        target = min(len(units),
                                     bias +
                                     done * len(units) // max(1, n_tiles - 4))
                        while emitted < target:
                            units[emitted]()
                            emitted += 1

                    # normalize: only the PSUM->SBUF copy now; the rest is
                    # deferred so it never head-of-line blocks PE
                    o_sb = smallpool.tile([65, 1024], BF16, tag="osb2",
                                          name="o_sb")
                    with nc.allow_low_precision(reason="bf16 O"):
                        nc.vector.tensor_copy(o_sb[:], o_ps[:])

                    final = (j == NCH - 1 and b == B - 1)

                    def norm_recip(o_sb=o_sb):
                        r_sb = smallpool.tile([1, 1024], BF16, tag="r",
                                              name="r_sb")
                        with nc.allow_low_precision(reason="denom"):
                            nc.vector.reciprocal(r_sb[:], o_sb[64:65, :])
                        return r_sb

                    def norm_rest(r_sb, b=b, j=j, o_sb=o_sb, final=final):
                        rb_ps = sps.tile([128, 1024], F32, tag="s_ps",
                                         name="rb_ps")
                        for h in range(HL):
                            # one matmul per PSUM bank (ISA: no cross-bank)
                            nc.tensor.matmul(
                                rb_ps[0:64, 512 * h:512 * (h + 1)], onesr,
                                r_sb[:, 512 * h:512 * (h + 1)],
                                start=True, stop=True)
                        ofin = ofinpool.tile([64, 1024], BF16, name="ofin")
                        if final:
                            # tail: skip the rb copy, read PSUM directly,
                            # and pipeline each head's store behind its mul
                            for h in range(HL):
                                sl = slice(512 * h, 512 * (h + 1))
                                with nc.allow_low_precision(reason="bf16 O"):
                                    nc.vector.tensor_mul(
                                        ofin[:, sl], o_sb[0:64, sl],
                                        rb_ps[0:64, sl])
                                nc.sync.dma_start(
                                    inb[b].ap()[2 * j:2 * j + 2,
                                                64 * h:64 * (h + 1), :]
                                    .rearrange("s p u -> p s u"),
                                    ofin[:, sl].rearrange(
                                        "p (s u) -> p s u", s=2))
                            return
                        else:
                            rb_sb = smallpool.tile([64, 1024], BF16,
                                                   tag="rb", name="rb_sb")
                            with nc.allow_low_precision(reason="bf16 rb"):
                                nc.vector.tensor_copy(rb_sb[:],
                                                      rb_ps[0:64, :])
                            with nc.allow_low_precision(reason="bf16 O"):
                                nc.vector.tensor_mul(ofin[:], o_sb[0:64, :],
                                                     rb_sb[:])
                        for h in range(HL):
                            nc.gpsimd.dma_start(
                                inb[b].ap()[2 * j:2 * j + 2,
                                            64 * h:64 * (h + 1), :]
                                .rearrange("s p u -> p s u"),
                                ofin[:, 512 * h:512 * (h + 1)].rearrange(
                                    "p (s u) -> p s u", s=2))

                    if final:
                        final_tail.append((norm_recip, norm_rest))
                    else:
                        pending.append(
                            lambda nr=norm_recip, ns=norm_rest: ns(nr()))
                while emitted < len(units):
                    units[emitted]()
                    emitted += 1

            def emit_a2a(b):
                if sim:
                    # stand-in with comparable cost for the cost-model sim
                    nc.sync.dma_start(outb[b].ap(), inb[b].ap())
                else:
                    nc.gpsimd.collective_compute(
                        "AllToAll", mybir.AluOpType.bypass,
                        replica_groups=[list(range(NCORES))],
                        ins=[inb[b].ap().opt()], outs=[outb[b].ap().opt()],
                    )

            # prologue: only batch-0 chunk-0 q/k/v/vt runs un-interleaved;
            # the rest of batch-0 qkv becomes front-biased filler
            u0 = qkv_units(0)
            for u in u0[:4]:
                u()
            reserved = []
            for b in range(B):
                units = qkv_units(b + 1) if b + 1 < B else []
                if b == 0:
                    units = u0[4:] + units
                op_units = []
                if b == B - 1:
                    # all deferred out-projections fill the last batch's
                    # otherwise exp-paced attention
                    for bb in range(B - 1):
                        op_units.extend(outproj_units(bb))
                    reserved = op_units[-8:]
                    op_units = op_units[:-8]
                units = interleave(units, op_units)

                def tail_fn(bb=b - 1):
                    emit_a2a(bb)
                    emit_ot_load(bb)

                emit_attn(b, units, tail_fn if b else None,
                          bias=3 if b == 0 else 0)
            nr, ns = final_tail.pop()
            r_sb_f = nr()       # reciprocal runs under the first unit
            if reserved:
                reserved[0]()
            ns(r_sb_f)
            emit_a2a(B - 1)
            emit_ot_load(B - 1, split=True)
            for u in reserved[1:]:
                u()
            for u in outproj_units(B - 1, store_eng=nc.sync, alt=True):
                u()
    nc.compile()
    return nc


def _get_nc():
    if "nc" not in _CACHE:
        _CACHE["nc"] = _build()
    return _CACHE["nc"]


def kernel(x, W_qkv, b_qkv, W_proj, b_proj):
    x = np.asarray(x, dtype=np.float32)
    W_qkv = np.asarray(W_qkv, dtype=np.float32)
    b_qkv = np.asarray(b_qkv, dtype=np.float32)
    W_proj = np.asarray(W_proj, dtype=np.float32)
    b_proj = np.asarray(b_proj, dtype=np.float32)

    BF = ml_dtypes.bfloat16
    scale = 1.0 / np.sqrt(HD)
    xt = np.ascontiguousarray(x.reshape(B * T, C).T).astype(BF)  # [C, B*T]
    wp = np.ascontiguousarray(W_proj).astype(BF)                 # [C, C]
    bp = np.ascontiguousarray(b_proj.reshape(NKC, 128).T)        # [128, 8]

    cn = np.zeros((128, CN_W), dtype=BF)
    cn[:, CN_ID:CN_ID + 128] = np.eye(128, dtype=np.float32)
    tril = (np.arange(128)[None, :] >= np.arange(128)[:, None])
    cn[:, CN_MASK:CN_MASK + 128] = tril.astype(np.float32)
    cn[:, CN_MASK + 128:CN_MASK + 256] = tril.astype(np.float32)
    cn[:, CN_ONES:CN_ONES + 32] = 1.0
    cn[0, CN_ONESR:CN_ONESR + 64] = 1.0

    qw = W_qkv[:, 0:C]
    kw = W_qkv[:, C:2 * C]
    vw = W_qkv[:, 2 * C:3 * C]
    qb, kb, vb = b_qkv[0:C], b_qkv[C:2 * C], b_qkv[2 * C:3 * C]

    in_maps = []
    for c in range(NCORES):
        cols = slice(2 * c * HD, (2 * c + 2) * HD)  # this core's 128 dims
        bq = np.stack([qb[cols] * scale, kb[cols], vb[cols]], axis=1)  # [128,3]
        in_maps.append({
            "xt": xt,
            "wq": np.ascontiguousarray(qw[:, cols] * scale).astype(BF),
            "wk": np.ascontiguousarray(kw[:, cols]).astype(BF),
            "wv": np.ascontiguousarray(vw[:, cols]).astype(BF),
            "wp": wp,
            "cn": cn,
            "bqkv": np.ascontiguousarray(bq),
            "bp": bp,
        })

    nc = _get_nc()
    _CACHE["last_in_maps"] = in_maps
    res = run_bass_kernel_spmd(nc, in_maps, core_ids=list(range(NCORES)))

    # outp[c]: [C, B*PIECE] (cols: b-major, then 256 tokens of piece c)
    allo = np.stack([np.asarray(res.results[c]["outp"], dtype=np.float32)
                     for c in range(NCORES)])
    allo = allo.reshape(NCORES, C, B, PIECE)       # [c, ch, b, u]
    out = allo.transpose(2, 0, 3, 1).reshape(B, T, C)
    return np.ascontiguousarray(out)


# revision 24
# speedup vs baseline: 1.0137x; 1.0137x over previous
"""Causal self-attention (B=4, T=2048, C=1024, H=16) on 8 trn2 NeuronCores.

Sharding: head-pair parallel. Core c owns heads {2c, 2c+1} for all 4 batches.
 - host: x pre-transposed to xT [C, B*T] bf16; W_qkv pre-sliced per core into
   wq/wk/wv [C, 128] bf16 (softmax scale folded into wq), W_proj bf16,
   f32 biases, and a packed bf16 constants tensor (identity, causal tril
   mask, ones) so the kernel needs no GpSimd compute ucode.
 - software pipeline: the attention tile loop of batch b doubles as the
   scheduler for everything else. qkv projections of batch b+1, the V
   transposes of b+1, and the out-projection of b-1 are emitted as small
   self-contained "filler units" between attention tiles, so the PE stays
   busy instead of pacing on ScalarE's exp. x tiles are prefetched two
   batches ahead ([128, 2048] per contraction chunk, one DMA each).
 - attention in S^T orientation: S^T[tk, tq] = kT.T@qT bf16 tiles
   [128, 512], diagonal tiles first. P^T = exp(S^T) on ScalarE (PSUM->SBUF
   bf16); the 128x128 causal wedge of diagonal tiles is zeroed on DVE by
   multiplying with a host-shipped 0/1 tril mask (exp of unmasked future
   logits is finite so this is safe). O-matmul lhsT = [v_h | ones] (M=65)
   gives O^T and the softmax denominator in one pass. Normalize: only the
   PSUM->SBUF copy happens at chunk end; reciprocal + ones-broadcast matmul
   + multiply + stores are deferred into the next chunk's tile loop.
 - per-batch AllToAll (512 KB/rank bf16) reshards O^T head->token shards.
   SBUF->DRAM stores go through the (otherwise idle) GpSimd DGE queue so
   they never head-of-line block the sync DGE queue that feeds loads and
   the collective. Column-parallel out-projection, bf16 out^T, host
   upcasts and reassembles.
"""
import numpy as np
import ml_dtypes
import concourse.bacc as bacc
import concourse.mybir as mybir
import concourse.tile as tile
from concourse.bass_utils import run_bass_kernel_spmd

F32 = mybir.dt.float32
BF16 = mybir.dt.bfloat16
Exp = mybir.ActivationFunctionType.Exp

NCORES = 8
B, T, C, H = 4, 2048, 1024, 16
HD = C // H          # 64
HL = H // NCORES     # 2 heads per core
D2 = HL * HD         # 128 rows of local head-pair dims
TB = T               # tokens per batch
NKC = C // 128       # 8 contraction chunks
NCH = TB // 512      # 4 tq chunks per batch
NTK = TB // 128      # 16 tk tiles per batch
PIECE = TB // NCORES  # 256 tokens per (batch, core) piece after AllToAll

# packed constants layout (bf16, [128, 480]):
#   cols 0:128  identity; 128:384 tril mask x2 heads; 384:416 ones (vh);
#   row 0 cols 416:480 ones (rb lhsT)
CN_ID = 0
CN_MASK = 128
CN_ONES = 384
CN_ONESR = 416
CN_W = 480

_CACHE = {}


def _build(sim=False):
    nc = bacc.Bacc("TRN2", target_bir_lowering=False, debug=False,
                   num_devices=1 if sim else NCORES)
    xt = nc.dram_tensor("xt", [C, B * T], BF16, kind="ExternalInput").ap()
    wq = nc.dram_tensor("wq", [C, D2], BF16, kind="ExternalInput").ap()
    wk = nc.dram_tensor("wk", [C, D2], BF16, kind="ExternalInput").ap()
    wv = nc.dram_tensor("wv", [C, D2], BF16, kind="ExternalInput").ap()
    wp = nc.dram_tensor("wp", [C, C], BF16, kind="ExternalInput").ap()
    cn = nc.dram_tensor("cn", [128, CN_W], BF16, kind="ExternalInput").ap()
    bqkv = nc.dram_tensor("bqkv", [D2, 3], F32, kind="ExternalInput").ap()
    bp = nc.dram_tensor("bp", [128, NKC], F32, kind="ExternalInput").ap()
    outp = nc.dram_tensor("outp", [C, B * PIECE], BF16,
                          kind="ExternalOutput").ap()

    inb = [nc.dram_tensor(f"inb{b}", [NCORES, D2, PIECE], BF16)
           for b in range(B)]
    outb = [nc.dram_tensor(f"outb{b}", [NCORES, D2, PIECE], BF16)
            for b in range(B)]

    with tile.TileContext(nc) as tc:
        with (
            tc.tile_pool(name="const", bufs=1) as cpool,
            tc.tile_pool(name="w", bufs=1) as wpool,
            tc.tile_pool(name="xt", bufs=16) as xpool,
            tc.tile_pool(name="qk", bufs=2) as qkpool,
            tc.tile_pool(name="vstg", bufs=1) as vstgpool,
            tc.tile_pool(name="vh", bufs=2) as vhpool,
            tc.tile_pool(name="pt", bufs=5) as ptpool,
            tc.tile_pool(name="small", bufs=3) as smallpool,
            tc.tile_pool(name="ofin", bufs=3) as ofinpool,
            tc.tile_pool(name="proj", bufs=3) as projpool,
            tc.tile_pool(name="otp", bufs=3) as otpool,
            tc.tile_pool(name="mm", bufs=1, space="PSUM") as mmps,
            tc.tile_pool(name="s", bufs=2, space="PSUM") as sps,
            tc.tile_pool(name="o", bufs=1, space="PSUM") as ops,
        ):
            # ---- constants / weights, ordered so batch-0 qkv starts asap
            wq_sb = wpool.tile([128, NKC, D2], BF16)
            wk_sb = wpool.tile([128, NKC, D2], BF16)
            wv_sb = wpool.tile([128, NKC, D2], BF16)
            nc.sync.dma_start(
                wq_sb[:], wq.rearrange("(kc p) m -> p kc m", p=128))

            xts = {}

            def emit_xt_loads(b, striped=False):
                xts[b] = [xpool.tile([128, TB], BF16, name="xtile")
                          for _ in range(NKC)]
                if striped:
                    # two half-tile DMAs per chunk so batch-0 compute can
                    # start after the first 8 arrive; wk/wv slot in between
                    for half in range(2):
                        for kc in range(NKC):
                            nc.sync.dma_start(
                                xts[b][kc][:, 1024 * half:1024 * (half + 1)],
                                xt[128 * kc:128 * (kc + 1),
                                   b * TB + 1024 * half:
                                   b * TB + 1024 * (half + 1)])
                        if half == 0:
                            for t, d in ((wk_sb, wk), (wv_sb, wv)):
                                nc.sync.dma_start(
                                    t[:],
                                    d.rearrange("(kc p) m -> p kc m", p=128))
                else:
                    for kc in range(NKC):
                        nc.sync.dma_start(
                            xts[b][kc][:],
                            xt[128 * kc:128 * (kc + 1), b * TB:(b + 1) * TB])

            bqkv_sb = cpool.tile([D2, 3], F32)
            nc.sync.dma_start(bqkv_sb[:], bqkv)
            cn_sb = cpool.tile([128, CN_W], BF16)
            nc.sync.dma_start(cn_sb[:], cn)
            emit_xt_loads(0, striped=True)
            identb = cn_sb[:, CN_ID:CN_ID + 128]
            maskb = cn_sb[:, CN_MASK:CN_MASK + 256].rearrange(
                "p (g c) -> p g c", g=HL)
            onesb = cn_sb[:, CN_ONES:CN_ONES + 32]
            onesr = cn_sb[0:1, CN_ONESR:CN_ONESR + 64]

            emit_xt_loads(1)
            wp_sb = wpool.tile([128, NKC, C], BF16)
            for g in range(4):
                nc.sync.dma_start(
                    wp_sb[:, 2 * g:2 * (g + 1), :],
                    wp[256 * g:256 * (g + 1), :].rearrange(
                        "(two p) m -> p two m", p=128))
            bp_sb = cpool.tile([128, NKC], F32)
            nc.sync.dma_start(bp_sb[:], bp)

            qT = {}
            kT = {}
            vh = {}
            vT = {}

            def qkv_units(b):
                """12 qkv matmul groups + 4 v-transpose groups for batch b."""
                qT[b] = qkpool.tile([D2, TB], BF16, tag="qT", name="qT")
                kT[b] = qkpool.tile([D2, TB], BF16, tag="kT", name="kT")
                vT[b] = vstgpool.tile([D2, TB], BF16, name="vT")
                units = []

                def qkv_group(n, col, w_sb, dst, b=b):
                    def emit():
                        ps = mmps.tile([128, 512], F32, tag="qps",
                                       name="ps")
                        for kc in range(NKC):
                            nc.tensor.matmul(
                                ps[:], w_sb[:, kc, :],
                                xts[b][kc][:, 512 * n:512 * (n + 1)],
                                start=(kc == 0), stop=(kc == NKC - 1))
                        with nc.allow_low_precision(reason="bf16 qkv"):
                            nc.vector.tensor_scalar_add(
                                dst[:, 512 * n:512 * (n + 1)], ps[:],
                                bqkv_sb[:, col:col + 1])
                        if n == NCH - 1 and col == 2:
                            del xts[b]
                    return emit

                def vt_group(tk0, b=b):
                    def emit():
                        # layout: vh[:, h*(NTK*65) + tk*65 + (0:64)] = v tile
                        if tk0 == 0:
                            vh[b] = vhpool.tile([128, HL * NTK * 65], BF16,
                                                tag="vh", name="vh")
                            with nc.allow_low_precision(reason="ones"):
                                nc.vector.tensor_copy(vh[b][:, 64::65],
                                                      onesb[:])
                        vt_ps = mmps.tile([128, 512], BF16, tag="vtps")
                        for i in range(4):
                            nc.tensor.transpose(
                                vt_ps[:, 128 * i:128 * (i + 1)],
                                vT[b][:, 128 * (tk0 + i):
                                      128 * (tk0 + i + 1)],
                                identb)
                        with nc.allow_low_precision(reason="bf16 v"):
                            nc.vector.tensor_copy(
                                vh[b][:].rearrange(
                                    "p (h tk c) -> p h tk c",
                                    h=HL, tk=NTK)[:, :, tk0:tk0 + 4, 0:64],
                                vt_ps[:].rearrange(
                                    "p (i h c) -> p h i c", i=4, h=HL))
                    return emit

                for n in range(NCH):
                    for col, (w_sb, dst) in enumerate(
                            ((wq_sb, qT[b]), (wk_sb, kT[b]),
                             (wv_sb, vT[b]))):
                        units.append(qkv_group(n, col, w_sb, dst))
                    units.append(vt_group(4 * n))
                return units

            ot = {}

            def emit_ot_load(b, split=False):
                ot[b] = otpool.tile([128, NCORES, PIECE], BF16, name="ot")
                if split:
                    for g in range(4):
                        nc.sync.dma_start(
                            ot[b][:, 2 * g:2 * (g + 1), :],
                            outb[b].ap()[2 * g:2 * (g + 1)].rearrange(
                                "s p u -> p s u"))
                else:
                    nc.sync.dma_start(
                        ot[b][:], outb[b].ap().rearrange("s p u -> p s u"))

            def outproj_units(b, store_eng=None, alt=False):
                """4 out-projection column-pair groups for batch b."""
                units = []

                def op_group(mc2, b=b):
                    def emit():
                        osb = projpool.tile([128, 2, PIECE], BF16,
                                            tag="osb", name="osb")
                        for sub in range(2):
                            mcol = 2 * mc2 + sub
                            # at the tail, alternate PSUM tags so groups
                            # double-buffer (keeps the PE p-state warm)
                            if alt and sub == 1:
                                pp = sps.tile([128, 1024], F32, tag="s_ps",
                                              name="pp")
                            else:
                                pp = mmps.tile([128, 512], F32, tag="qps",
                                               name="pp")
                            for s8 in range(NCORES):
                                nc.tensor.matmul(
                                    pp[:, 0:PIECE],
                                    wp_sb[:, s8,
                                          128 * mcol:128 * (mcol + 1)],
                                    ot[b][:, s8, :], start=(s8 == 0),
                                    stop=(s8 == NCORES - 1))
                            with nc.allow_low_precision(reason="bf16 out"):
                                nc.vector.tensor_scalar_add(
                                    osb[:, sub, :], pp[:, 0:PIECE],
                                    bp_sb[:, mcol:mcol + 1])
                            if store_eng is not None:
                                store_eng.dma_start(
                                    outp[256 * mc2 + 128 * sub:
                                         256 * mc2 + 128 * (sub + 1),
                                         PIECE * b:PIECE * (b + 1)],
                                    osb[:, sub, :])
                        if store_eng is None:
                            nc.gpsimd.dma_start(
                                outp[256 * mc2:256 * (mc2 + 1),
                                     PIECE * b:PIECE * (b + 1)].rearrange(
                                         "(two p) u -> p two u", p=128),
                                osb[:])
                    return emit

                for mc2 in range(NKC // 2):
                    units.append(op_group(mc2))
                return units

            def interleave(a, bl):
                """a with elements of bl spread through (a leads)."""
                if not bl:
                    return list(a)
                out = []
                step = max(1, len(a) // (len(bl) + 1))
                bi = 0
                for i, u in enumerate(a):
                    out.append(u)
                    if bi < len(bl) and (i + 1) % step == 0 and i >= 2:
                        out.append(bl[bi])
                        bi += 1
                out.extend(bl[bi:])
                return out

            pending = []
            final_tail = []

            def flush_pending():
                while pending:
                    pending.pop(0)()

            def emit_attn(b, units, tail_fn=None, bias=0):
                if b + 2 < B:
                    emit_xt_loads(b + 2)
                n_tiles = sum(4 * j + 4 for j in range(NCH))
                done = 0
                emitted = 0
                for j in range(NCH):
                    o_ps = ops.tile([65, 1024], F32, tag="o")
                    # full tiles first (no mask dependency) so the chunk
                    # opener never waits on the o_ps WAR or DVE masks; the
                    # first tile always spans the full width (m=0 when j=0)
                    seq = list(range(4 * j)) + list(range(4 * j, 4 * j + 4))

                    def emit_o(tk, z, pt, idx, b=b, j=j, o_ps=o_ps,
                               last=False):
                        for h in range(HL):
                            nc.tensor.matmul(
                                o_ps[0:65, 512 * h + z:512 * (h + 1)],
                                vh[b][:, (h * NTK + tk) * 65:
                                      (h * NTK + tk + 1) * 65],
                                pt[:, 512 * h + z:512 * (h + 1)],
                                start=(idx == 0), stop=last)

                    prev = None
                    for idx, tk in enumerate(seq):
                        m = tk - 4 * j
                        z = 128 * m if m >= 0 else 0
                        s_ps = sps.tile([128, 1024], F32, tag="s_ps",
                                        name="s_ps")
                        for h in range(HL):
                            nc.tensor.matmul(
                                s_ps[:, 512 * h + z:512 * (h + 1)],
                                kT[b][64 * h:64 * (h + 1),
                                      128 * tk:128 * (tk + 1)],
                                qT[b][64 * h:64 * (h + 1),
                                      512 * j + z:512 * (j + 1)],
                                start=True, stop=True)
                        pt = ptpool.tile([128, 1024], BF16, tag="pt")
                        if z:
                            exp_src = s_ps[:].rearrange(
                                "p (g c) -> p g c", g=2)[:, :, z:]
                            exp_dst = pt[:].rearrange(
                                "p (g c) -> p g c", g=2)[:, :, z:]
                            nc.scalar.activation(exp_dst, exp_src, Exp)
                        else:
                            nc.scalar.activation(pt[:], s_ps[:], Exp)
                        if m >= 0:
                            # zero the causal wedge: keep where tq_l >= tk_l
                            ptv = pt[:].rearrange(
                                "p (g c) -> p g c", g=2)[:, :, z:z + 128]
                            with nc.allow_low_precision(reason="mask"):
                                nc.vector.tensor_mul(ptv, ptv, maskb)
                        # one-stage skew: emit O for the PREVIOUS tile so
                        # its exp has a full S-tile of latency slack
                        if prev is not None:
                            emit_o(*prev)
                        prev = (tk, z, pt, idx)
                        done += 1
                        if idx == 2:
                            flush_pending()
                            if j == 0 and tail_fn is not None:
                                tail_fn()
                                tail_fn = None
                        # pace filler so it finishes ~4 tiles before the end
                        target = min(len(units),
                                     bias +
                                     done * len(units) // max(1, n_tiles - 4))
                        while emitted < target:
                            units[emitted]()
                            emitted += 1
                    emit_o(*prev, last=True)

                    # normalize: only the PSUM->SBUF copy now; the rest is
                    # deferred so it never head-of-line blocks PE
                    o_sb = smallpool.tile([65, 1024], BF16, tag="osb2",
                                          name="o_sb")
                    with nc.allow_low_precision(reason="bf16 O"):
                        nc.vector.tensor_copy(o_sb[:], o_ps[:])

                    final = (j == NCH - 1 and b == B - 1)

                    def norm_recip(o_sb=o_sb):
                        r_sb = smallpool.tile([1, 1024], BF16, tag="r",
                                              name="r_sb")
                        with nc.allow_low_precision(reason="denom"):
                            nc.vector.reciprocal(r_sb[:], o_sb[64:65, :])
                        return r_sb

                    def norm_rest(r_sb, b=b, j=j, o_sb=o_sb, final=final):
                        rb_ps = sps.tile([128, 1024], F32, tag="s_ps",
                                         name="rb_ps")
                        for h in range(HL):
                            # one matmul per PSUM bank (ISA: no cross-bank)
                            nc.tensor.matmul(
                                rb_ps[0:64, 512 * h:512 * (h + 1)], onesr,
                                r_sb[:, 512 * h:512 * (h + 1)],
                                start=True, stop=True)
                        ofin = ofinpool.tile([64, 1024], BF16, name="ofin")
                        if final:
                            # tail: skip the rb copy, read PSUM directly,
                            # and pipeline each head's store behind its mul
                            for h in range(HL):
                                sl = slice(512 * h, 512 * (h + 1))
                                with nc.allow_low_precision(reason="bf16 O"):
                                    nc.vector.tensor_mul(
                                        ofin[:, sl], o_sb[0:64, sl],
                                        rb_ps[0:64, sl])
                                nc.sync.dma_start(
                                    inb[b].ap()[2 * j:2 * j + 2,
                                                64 * h:64 * (h + 1), :]
                                    .rearrange("s p u -> p s u"),
                                    ofin[:, sl].rearrange(
                                        "p (s u) -> p s u", s=2))
                            return
                        else:
                            rb_sb = smallpool.tile([64, 1024], BF16,
                                                   tag="rb", name="rb_sb")
                            with nc.allow_low_precision(reason="bf16 rb"):
                                nc.vector.tensor_copy(rb_sb[:],
                                                      rb_ps[0:64, :])
                            with nc.allow_low_precision(reason="bf16 O"):
                                nc.vector.tensor_mul(ofin[:], o_sb[0:64, :],
                                                     rb_sb[:])
                        for h in range(HL):
                            nc.gpsimd.dma_start(
                                inb[b].ap()[2 * j:2 * j + 2,
                                            64 * h:64 * (h + 1), :]
                                .rearrange("s p u -> p s u"),
                                ofin[:, 512 * h:512 * (h + 1)].rearrange(
                                    "p (s u) -> p s u", s=2))

                    if final:
                        final_tail.append((norm_recip, norm_rest))
                    else:
                        pending.append(
                            lambda nr=norm_recip, ns=norm_rest: ns(nr()))
                while emitted < len(units):
                    units[emitted]()
                    emitted += 1

            def emit_a2a(b):
                if sim:
                    # stand-in with comparable cost for the cost-model sim
                    nc.sync.dma_start(outb[b].ap(), inb[b].ap())
                else:
                    nc.gpsimd.collective_compute(
                        "AllToAll", mybir.AluOpType.bypass,
                        replica_groups=[list(range(NCORES))],
                        ins=[inb[b].ap().opt()], outs=[outb[b].ap().opt()],
                    )

            # prologue: only batch-0 chunk-0 q/k/v/vt runs un-interleaved;
            # the rest of batch-0 qkv becomes front-biased filler
            u0 = qkv_units(0)
            for u in u0[:4]:
                u()
            reserved = []
            for b in range(B):
                units = qkv_units(b + 1) if b + 1 < B else []
                if b == 0:
                    units = u0[4:] + units
                op_units = []
                if b == B - 1:
                    # all deferred out-projections fill the last batch's
                    # otherwise exp-paced attention
                    for bb in range(B - 1):
                        op_units.extend(outproj_units(bb))
                    reserved = op_units[-6:]
                    op_units = op_units[:-6]
                units = interleave(units, op_units)

                def tail_fn(bb=b - 1):
                    emit_a2a(bb)
                    emit_ot_load(bb)

                emit_attn(b, units, tail_fn if b else None,
                          bias=3 if b == 0 else 0)
            nr, ns = final_tail.pop()
            r_sb_f = nr()       # reciprocal runs under the first unit
            if reserved:
                reserved[0]()
            ns(r_sb_f)
            emit_a2a(B - 1)
            emit_ot_load(B - 1, split=True)
            for u in reserved[1:]:
                u()
            for u in outproj_units(B - 1, store_eng=nc.sync, alt=True):
                u()
    nc.compile()
    return nc


def _get_nc():
    if "nc" not in _CACHE:
        _CACHE["nc"] = _build()
    return _CACHE["nc"]


def kernel(x, W_qkv, b_qkv, W_proj, b_proj):
    x = np.asarray(x, dtype=np.float32)
    W_qkv = np.asarray(W_qkv, dtype=np.float32)
    b_qkv = np.asarray(b_qkv, dtype=np.float32)
    W_proj = np.asarray(W_proj, dtype=np.float32)
    b_proj = np.asarray(b_proj, dtype=np.float32)

    BF = ml_dtypes.bfloat16
    scale = 1.0 / np.sqrt(HD)
    xt = np.ascontiguousarray(x.reshape(B * T, C).T).astype(BF)  # [C, B*T]
    wp = np.ascontiguousarray(W_proj).astype(BF)                 # [C, C]
    bp = np.ascontiguousarray(b_proj.reshape(NKC, 128).T)        # [128, 8]

    cn = np.zeros((128, CN_W), dtype=BF)
    cn[:, CN_ID:CN_ID + 128] = np.eye(128, dtype=np.float32)
    tril = (np.arange(128)[None, :] >= np.arange(128)[:, None])
    cn[:, CN_MASK:CN_MASK + 128] = tril.astype(np.float32)
    cn[:, CN_MASK + 128:CN_MASK + 256] = tril.astype(np.float32)
    cn[:, CN_ONES:CN_ONES + 32] = 1.0
    cn[0, CN_ONESR:CN_ONESR + 64] = 1.0

    qw = W_qkv[:, 0:C]
    kw = W_qkv[:, C:2 * C]
    vw = W_qkv[:, 2 * C:3 * C]
    qb, kb, vb = b_qkv[0:C], b_qkv[C:2 * C], b_qkv[2 * C:3 * C]

    in_maps = []
    for c in range(NCORES):
        cols = slice(2 * c * HD, (2 * c + 2) * HD)  # this core's 128 dims
        bq = np.stack([qb[cols] * scale, kb[cols], vb[cols]], axis=1)  # [128,3]
        in_maps.append({
            "xt": xt,
            "wq": np.ascontiguousarray(qw[:, cols] * scale).astype(BF),
            "wk": np.ascontiguousarray(kw[:, cols]).astype(BF),
            "wv": np.ascontiguousarray(vw[:, cols]).astype(BF),
            "wp": wp,
            "cn": cn,
            "bqkv": np.ascontiguousarray(bq),
            "bp": bp,
        })

    nc = _get_nc()
    _CACHE["last_in_maps"] = in_maps
    res = run_bass_kernel_spmd(nc, in_maps, core_ids=list(range(NCORES)))

    # outp[c]: [C, B*PIECE] (cols: b-major, then 256 tokens of piece c)
    allo = np.stack([np.asarray(res.results[c]["outp"], dtype=np.float32)
                     for c in range(NCORES)])
    allo = allo.reshape(NCORES, C, B, PIECE)       # [c, ch, b, u]
    out = allo.transpose(2, 0, 3, 1).reshape(B, T, C)
    return np.ascontiguousarray(out)


# revision 25
# speedup vs baseline: 1.0177x; 1.0040x over previous
"""Causal self-attention (B=4, T=2048, C=1024, H=16) on 8 trn2 NeuronCores.

Sharding: head-pair parallel. Core c owns heads {2c, 2c+1} for all 4 batches.
 - host: x pre-transposed to xT [C, B*T] bf16; W_qkv pre-sliced per core into
   wq/wk/wv [C, 128] bf16 (softmax scale folded into wq), W_proj bf16,
   f32 biases, and a packed bf16 constants tensor (identity, causal tril
   mask, ones) so the kernel needs no GpSimd compute ucode.
 - software pipeline: the attention tile loop of batch b doubles as the
   scheduler for everything else. qkv projections of batch b+1, the V
   transposes of b+1, and the out-projection of b-1 are emitted as small
   self-contained "filler units" between attention tiles, so the PE stays
   busy instead of pacing on ScalarE's exp. x tiles are prefetched two
   batches ahead ([128, 2048] per contraction chunk, one DMA each).
 - attention in S^T orientation: S^T[tk, tq] = kT.T@qT bf16 tiles
   [128, 512], diagonal tiles first. P^T = exp(S^T) on ScalarE (PSUM->SBUF
   bf16); the 128x128 causal wedge of diagonal tiles is zeroed on DVE by
   multiplying with a host-shipped 0/1 tril mask (exp of unmasked future
   logits is finite so this is safe). O-matmul lhsT = [v_h | ones] (M=65)
   gives O^T and the softmax denominator in one pass. Normalize: only the
   PSUM->SBUF copy happens at chunk end; reciprocal + ones-broadcast matmul
   + multiply + stores are deferred into the next chunk's tile loop.
 - per-batch AllToAll (512 KB/rank bf16) reshards O^T head->token shards.
   SBUF->DRAM stores go through the (otherwise idle) GpSimd DGE queue so
   they never head-of-line block the sync DGE queue that feeds loads and
   the collective. Column-parallel out-projection, bf16 out^T, host
   upcasts and reassembles.
"""
import numpy as np
import ml_dtypes
import concourse.bacc as bacc
import concourse.mybir as mybir
import concourse.tile as tile
from concourse.bass_utils import run_bass_kernel_spmd

F32 = mybir.dt.float32
BF16 = mybir.dt.bfloat16
Exp = mybir.ActivationFunctionType.Exp

NCORES = 8
B, T, C, H = 4, 2048, 1024, 16
HD = C // H          # 64
HL = H // NCORES     # 2 heads per core
D2 = HL * HD         # 128 rows of local head-pair dims
TB = T               # tokens per batch
NKC = C // 128       # 8 contraction chunks
NCH = TB // 512      # 4 tq chunks per batch
NTK = TB // 128      # 16 tk tiles per batch
PIECE = TB // NCORES  # 256 tokens per (batch, core) piece after AllToAll

# packed constants layout (bf16, [128, 480]):
#   cols 0:128  identity; 128:384 tril mask x2 heads; 384:416 ones (vh);
#   row 0 cols 416:480 ones (rb lhsT)
CN_ID = 0
CN_MASK = 128
CN_ONES = 384
CN_ONESR = 416
CN_W = 480

_CACHE = {}


def _build(sim=False):
    nc = bacc.Bacc("TRN2", target_bir_lowering=False, debug=False,
                   num_devices=1 if sim else NCORES)
    xt = nc.dram_tensor("xt", [C, B * T], BF16, kind="ExternalInput").ap()
    wq = nc.dram_tensor("wq", [C, D2], BF16, kind="ExternalInput").ap()
    wk = nc.dram_tensor("wk", [C, D2], BF16, kind="ExternalInput").ap()
    wv = nc.dram_tensor("wv", [C, D2], BF16, kind="ExternalInput").ap()
    wp = nc.dram_tensor("wp", [C, C], BF16, kind="ExternalInput").ap()
    cn = nc.dram_tensor("cn", [128, CN_W], BF16, kind="ExternalInput").ap()
    bqkv = nc.dram_tensor("bqkv", [D2, 3], F32, kind="ExternalInput").ap()
    bp = nc.dram_tensor("bp", [128, NKC], F32, kind="ExternalInput").ap()
    outp = nc.dram_tensor("outp", [C, B * PIECE], BF16,
                          kind="ExternalOutput").ap()

    inb = [nc.dram_tensor(f"inb{b}", [NCORES, D2, PIECE], BF16)
           for b in range(B)]
    outb = [nc.dram_tensor(f"outb{b}", [NCORES, D2, PIECE], BF16)
            for b in range(B)]

    with tile.TileContext(nc) as tc:
        with (
            tc.tile_pool(name="const", bufs=1) as cpool,
            tc.tile_pool(name="w", bufs=1) as wpool,
            tc.tile_pool(name="xt", bufs=16) as xpool,
            tc.tile_pool(name="qk", bufs=2) as qkpool,
            tc.tile_pool(name="vstg", bufs=1) as vstgpool,
            tc.tile_pool(name="vh", bufs=2) as vhpool,
            tc.tile_pool(name="pt", bufs=5) as ptpool,
            tc.tile_pool(name="small", bufs=3) as smallpool,
            tc.tile_pool(name="ofin", bufs=3) as ofinpool,
            tc.tile_pool(name="proj", bufs=3) as projpool,
            tc.tile_pool(name="otp", bufs=3) as otpool,
            tc.tile_pool(name="mm", bufs=1, space="PSUM") as mmps,
            tc.tile_pool(name="s", bufs=2, space="PSUM") as sps,
            tc.tile_pool(name="o", bufs=1, space="PSUM") as ops,
        ):
            # ---- constants / weights, ordered so batch-0 qkv starts asap
            wq_sb = wpool.tile([128, NKC, D2], BF16)
            wk_sb = wpool.tile([128, NKC, D2], BF16)
            wv_sb = wpool.tile([128, NKC, D2], BF16)
            nc.sync.dma_start(
                wq_sb[:], wq.rearrange("(kc p) m -> p kc m", p=128))

            xts = {}

            def emit_xt_loads(b, striped=False):
                xts[b] = [xpool.tile([128, TB], BF16, name="xtile")
                          for _ in range(NKC)]
                if striped:
                    # two half-tile DMAs per chunk so batch-0 compute can
                    # start after the first 8 arrive; wk/wv slot in between
                    for half in range(2):
                        for kc in range(NKC):
                            nc.sync.dma_start(
                                xts[b][kc][:, 1024 * half:1024 * (half + 1)],
                                xt[128 * kc:128 * (kc + 1),
                                   b * TB + 1024 * half:
                                   b * TB + 1024 * (half + 1)])
                        if half == 0:
                            for t, d in ((wk_sb, wk), (wv_sb, wv)):
                                nc.sync.dma_start(
                                    t[:],
                                    d.rearrange("(kc p) m -> p kc m", p=128))
                else:
                    for kc in range(NKC):
                        nc.sync.dma_start(
                            xts[b][kc][:],
                            xt[128 * kc:128 * (kc + 1), b * TB:(b + 1) * TB])

            bqkv_sb = cpool.tile([D2, 3], F32)
            nc.sync.dma_start(bqkv_sb[:], bqkv)
            cn_sb = cpool.tile([128, CN_W], BF16)
            nc.sync.dma_start(cn_sb[:], cn)
            emit_xt_loads(0, striped=True)
            identb = cn_sb[:, CN_ID:CN_ID + 128]
            maskb = cn_sb[:, CN_MASK:CN_MASK + 256].rearrange(
                "p (g c) -> p g c", g=HL)
            onesb = cn_sb[:, CN_ONES:CN_ONES + 32]
            onesr = cn_sb[0:1, CN_ONESR:CN_ONESR + 64]

            emit_xt_loads(1)
            wp_sb = wpool.tile([128, NKC, C], BF16)
            for g in range(4):
                nc.sync.dma_start(
                    wp_sb[:, 2 * g:2 * (g + 1), :],
                    wp[256 * g:256 * (g + 1), :].rearrange(
                        "(two p) m -> p two m", p=128))
            bp_sb = cpool.tile([128, NKC], F32)
            nc.sync.dma_start(bp_sb[:], bp)

            qT = {}
            kT = {}
            vh = {}
            vT = {}

            def qkv_units(b):
                """12 qkv matmul groups + 4 v-transpose groups for batch b."""
                qT[b] = qkpool.tile([D2, TB], BF16, tag="qT", name="qT")
                kT[b] = qkpool.tile([D2, TB], BF16, tag="kT", name="kT")
                vT[b] = vstgpool.tile([D2, TB], BF16, name="vT")
                units = []

                def qkv_group(n, col, w_sb, dst, b=b):
                    def emit():
                        ps = mmps.tile([128, 512], F32, tag="qps",
                                       name="ps")
                        for kc in range(NKC):
                            nc.tensor.matmul(
                                ps[:], w_sb[:, kc, :],
                                xts[b][kc][:, 512 * n:512 * (n + 1)],
                                start=(kc == 0), stop=(kc == NKC - 1))
                        with nc.allow_low_precision(reason="bf16 qkv"):
                            nc.vector.tensor_scalar_add(
                                dst[:, 512 * n:512 * (n + 1)], ps[:],
                                bqkv_sb[:, col:col + 1])
                        if n == NCH - 1 and col == 2:
                            del xts[b]
                    return emit

                def vt_group(tk0, b=b):
                    def emit():
                        # layout: vh[:, h*(NTK*65) + tk*65 + (0:64)] = v tile
                        if tk0 == 0:
                            vh[b] = vhpool.tile([128, HL * NTK * 65], BF16,
                                                tag="vh", name="vh")
                            with nc.allow_low_precision(reason="ones"):
                                nc.vector.tensor_copy(vh[b][:, 64::65],
                                                      onesb[:])
                        vt_ps = mmps.tile([128, 512], BF16, tag="vtps")
                        for i in range(4):
                            nc.tensor.transpose(
                                vt_ps[:, 128 * i:128 * (i + 1)],
                                vT[b][:, 128 * (tk0 + i):
                                      128 * (tk0 + i + 1)],
                                identb)
                        with nc.allow_low_precision(reason="bf16 v"):
                            nc.vector.tensor_copy(
                                vh[b][:].rearrange(
                                    "p (h tk c) -> p h tk c",
                                    h=HL, tk=NTK)[:, :, tk0:tk0 + 4, 0:64],
                                vt_ps[:].rearrange(
                                    "p (i h c) -> p h i c", i=4, h=HL))
                    return emit

                for n in range(NCH):
                    for col, (w_sb, dst) in enumerate(
                            ((wq_sb, qT[b]), (wk_sb, kT[b]),
                             (wv_sb, vT[b]))):
                        units.append(qkv_group(n, col, w_sb, dst))
                    units.append(vt_group(4 * n))
                return units

            ot = {}

            def emit_ot_load(b, split=False):
                ot[b] = otpool.tile([128, NCORES, PIECE], BF16, name="ot")
                if split:
                    for g in range(4):
                        nc.sync.dma_start(
                            ot[b][:, 2 * g:2 * (g + 1), :],
                            outb[b].ap()[2 * g:2 * (g + 1)].rearrange(
                                "s p u -> p s u"))
                else:
                    nc.sync.dma_start(
                        ot[b][:], outb[b].ap().rearrange("s p u -> p s u"))

            def outproj_units(b, store_eng=None, alt=False):
                """4 out-projection column-pair groups for batch b."""
                units = []

                def op_group(mc2, b=b):
                    def emit():
                        osb = projpool.tile([128, 2, PIECE], BF16,
                                            tag="osb", name="osb")
                        for sub in range(2):
                            mcol = 2 * mc2 + sub
                            # at the tail, alternate PSUM tags so groups
                            # double-buffer (keeps the PE p-state warm)
                            if alt and sub == 1:
                                pp = sps.tile([128, 1024], F32, tag="s_ps",
                                              name="pp")
                            else:
                                pp = mmps.tile([128, 512], F32, tag="qps",
                                               name="pp")
                            for s8 in range(NCORES):
                                nc.tensor.matmul(
                                    pp[:, 0:PIECE],
                                    wp_sb[:, s8,
                                          128 * mcol:128 * (mcol + 1)],
                                    ot[b][:, s8, :], start=(s8 == 0),
                                    stop=(s8 == NCORES - 1))
                            with nc.allow_low_precision(reason="bf16 out"):
                                nc.vector.tensor_scalar_add(
                                    osb[:, sub, :], pp[:, 0:PIECE],
                                    bp_sb[:, mcol:mcol + 1])
                            if store_eng is not None:
                                store_eng.dma_start(
                                    outp[256 * mc2 + 128 * sub:
                                         256 * mc2 + 128 * (sub + 1),
                                         PIECE * b:PIECE * (b + 1)],
                                    osb[:, sub, :])
                        if store_eng is None:
                            nc.gpsimd.dma_start(
                                outp[256 * mc2:256 * (mc2 + 1),
                                     PIECE * b:PIECE * (b + 1)].rearrange(
                                         "(two p) u -> p two u", p=128),
                                osb[:])
                    return emit

                for mc2 in range(NKC // 2):
                    units.append(op_group(mc2))
                return units

            def interleave(a, bl):
                """a with elements of bl spread through (a leads)."""
                if not bl:
                    return list(a)
                out = []
                step = max(1, len(a) // (len(bl) + 1))
                bi = 0
                for i, u in enumerate(a):
                    out.append(u)
                    if bi < len(bl) and (i + 1) % step == 0 and i >= 2:
                        out.append(bl[bi])
                        bi += 1
                out.extend(bl[bi:])
                return out

            pending = []
            final_tail = []

            def flush_pending():
                while pending:
                    pending.pop(0)()

            def emit_attn(b, units, tail_fn=None, bias=0):
                if b + 2 < B:
                    emit_xt_loads(b + 2)
                n_tiles = sum(4 * j + 4 for j in range(NCH))
                done = 0
                emitted = 0
                for j in range(NCH):
                    o_ps = ops.tile([65, 1024], F32, tag="o")
                    # full tiles first (no mask dependency) so the chunk
                    # opener never waits on the o_ps WAR or DVE masks; the
                    # first tile always spans the full width (m=0 when j=0)
                    seq = list(range(4 * j)) + list(range(4 * j, 4 * j + 4))

                    def emit_o(tk, z, pt, idx, b=b, j=j, o_ps=o_ps,
                               last=False):
                        for h in range(HL):
                            nc.tensor.matmul(
                                o_ps[0:65, 512 * h + z:512 * (h + 1)],
                                vh[b][:, (h * NTK + tk) * 65:
                                      (h * NTK + tk + 1) * 65],
                                pt[:, 512 * h + z:512 * (h + 1)],
                                start=(idx == 0), stop=last)

                    prev = []
                    for idx, tk in enumerate(seq):
                        m = tk - 4 * j
                        z = 128 * m if m >= 0 else 0
                        s_ps = sps.tile([128, 1024], F32, tag="s_ps",
                                        name="s_ps")
                        for h in range(HL):
                            nc.tensor.matmul(
                                s_ps[:, 512 * h + z:512 * (h + 1)],
                                kT[b][64 * h:64 * (h + 1),
                                      128 * tk:128 * (tk + 1)],
                                qT[b][64 * h:64 * (h + 1),
                                      512 * j + z:512 * (j + 1)],
                                start=True, stop=True)
                        pt = ptpool.tile([128, 1024], BF16, tag="pt")
                        if z:
                            exp_src = s_ps[:].rearrange(
                                "p (g c) -> p g c", g=2)[:, :, z:]
                            exp_dst = pt[:].rearrange(
                                "p (g c) -> p g c", g=2)[:, :, z:]
                            nc.scalar.activation(exp_dst, exp_src, Exp)
                        else:
                            nc.scalar.activation(pt[:], s_ps[:], Exp)
                        if m >= 0:
                            # zero the causal wedge: keep where tq_l >= tk_l
                            ptv = pt[:].rearrange(
                                "p (g c) -> p g c", g=2)[:, :, z:z + 128]
                            with nc.allow_low_precision(reason="mask"):
                                nc.vector.tensor_mul(ptv, ptv, maskb)
                        # two-stage skew: emit O two tiles behind so each
                        # exp has two S-tiles of latency slack
                        if len(prev) == 2:
                            emit_o(*prev.pop(0))
                        prev.append((tk, z, pt, idx))
                        done += 1
                        if idx == 2:
                            flush_pending()
                            if j == 0 and tail_fn is not None:
                                tail_fn()
                                tail_fn = None
                        # pace filler so it finishes ~4 tiles before the end
                        pace_end = n_tiles + 4 if b == B - 1 else n_tiles - 4
                        target = min(len(units),
                                     bias +
                                     done * len(units) // max(1, pace_end))
                        while emitted < target:
                            units[emitted]()
                            emitted += 1
                    for pi, pv in enumerate(prev):
                        emit_o(*pv, last=(pi == len(prev) - 1))

                    # normalize: only the PSUM->SBUF copy now; the rest is
                    # deferred so it never head-of-line blocks PE
                    o_sb = smallpool.tile([65, 1024], BF16, tag="osb2",
                                          name="o_sb")
                    with nc.allow_low_precision(reason="bf16 O"):
                        nc.vector.tensor_copy(o_sb[:], o_ps[:])

                    final = (j == NCH - 1 and b == B - 1)

                    def norm_recip(o_sb=o_sb):
                        r_sb = smallpool.tile([1, 1024], BF16, tag="r",
                                              name="r_sb")
                        with nc.allow_low_precision(reason="denom"):
                            nc.vector.reciprocal(r_sb[:], o_sb[64:65, :])
                        return r_sb

                    def norm_rest(r_sb, b=b, j=j, o_sb=o_sb, final=final):
                        rb_ps = sps.tile([128, 1024], F32, tag="s_ps",
                                         name="rb_ps")
                        for h in range(HL):
                            # one matmul per PSUM bank (ISA: no cross-bank)
                            nc.tensor.matmul(
                                rb_ps[0:64, 512 * h:512 * (h + 1)], onesr,
                                r_sb[:, 512 * h:512 * (h + 1)],
                                start=True, stop=True)
                        ofin = ofinpool.tile([64, 1024], BF16, name="ofin")
                        if final:
                            # tail: skip the rb copy, read PSUM directly,
                            # and pipeline each head's store behind its mul
                            for h in range(HL):
                                sl = slice(512 * h, 512 * (h + 1))
                                with nc.allow_low_precision(reason="bf16 O"):
                                    nc.vector.tensor_mul(
                                        ofin[:, sl], o_sb[0:64, sl],
                                        rb_ps[0:64, sl])
                                nc.sync.dma_start(
                                    inb[b].ap()[2 * j:2 * j + 2,
                                                64 * h:64 * (h + 1), :]
                                    .rearrange("s p u -> p s u"),
                                    ofin[:, sl].rearrange(
                                        "p (s u) -> p s u", s=2))
                            return
                        else:
                            rb_sb = smallpool.tile([64, 1024], BF16,
                                                   tag="rb", name="rb_sb")
                            with nc.allow_low_precision(reason="bf16 rb"):
                                nc.vector.tensor_copy(rb_sb[:],
                                                      rb_ps[0:64, :])
                            with nc.allow_low_precision(reason="bf16 O"):
                                nc.vector.tensor_mul(ofin[:], o_sb[0:64, :],
                                                     rb_sb[:])
                        for h in range(HL):
                            nc.gpsimd.dma_start(
                                inb[b].ap()[2 * j:2 * j + 2,
                                            64 * h:64 * (h + 1), :]
                                .rearrange("s p u -> p s u"),
                                ofin[:, 512 * h:512 * (h + 1)].rearrange(
                                    "p (s u) -> p s u", s=2))

                    if final:
                        final_tail.append((norm_recip, norm_rest))
                    else:
                        pending.append(
                            lambda nr=norm_recip, ns=norm_rest: ns(nr()))
                while emitted < len(units):
                    units[emitted]()
                    emitted += 1

            def emit_a2a(b):
                if sim:
                    # stand-in with comparable cost for the cost-model sim
                    nc.sync.dma_start(outb[b].ap(), inb[b].ap())
                else:
                    nc.gpsimd.collective_compute(
                        "AllToAll", mybir.AluOpType.bypass,
                        replica_groups=[list(range(NCORES))],
                        ins=[inb[b].ap().opt()], outs=[outb[b].ap().opt()],
                    )

            # prologue: only batch-0 chunk-0 q/k/v/vt runs un-interleaved;
            # the rest of batch-0 qkv becomes front-biased filler
            u0 = qkv_units(0)
            for u in u0[:4]:
                u()
            reserved = []
            for b in range(B):
                units = qkv_units(b + 1) if b + 1 < B else []
                if b == 0:
                    units = u0[4:] + units
                op_units = []
                if b == B - 1:
                    # all deferred out-projections fill the last batch's
                    # otherwise exp-paced attention
                    for bb in range(B - 1):
                        op_units.extend(outproj_units(bb))
                    reserved = op_units[-6:]
                    op_units = op_units[:-6]
                units = interleave(units, op_units)

                def tail_fn(bb=b - 1):
                    emit_a2a(bb)
                    emit_ot_load(bb)

                emit_attn(b, units, tail_fn if b else None,
                          bias=3 if b == 0 else 0)
            nr, ns = final_tail.pop()
            r_sb_f = nr()       # reciprocal runs under the first unit
            if reserved:
                reserved[0]()
            ns(r_sb_f)
            emit_a2a(B - 1)
            emit_ot_load(B - 1, split=True)
            for u in reserved[1:]:
                u()
            for u in outproj_units(B - 1, store_eng=nc.sync, alt=True):
                u()
    nc.compile()
    return nc


def _get_nc():
    if "nc" not in _CACHE:
        _CACHE["nc"] = _build()
    return _CACHE["nc"]


def kernel(x, W_qkv, b_qkv, W_proj, b_proj):
    x = np.asarray(x, dtype=np.float32)
    W_qkv = np.asarray(W_qkv, dtype=np.float32)
    b_qkv = np.asarray(b_qkv, dtype=np.float32)
    W_proj = np.asarray(W_proj, dtype=np.float32)
    b_proj = np.asarray(b_proj, dtype=np.float32)

    BF = ml_dtypes.bfloat16
    scale = 1.0 / np.sqrt(HD)
    xt = np.ascontiguousarray(x.reshape(B * T, C).T).astype(BF)  # [C, B*T]
    wp = np.ascontiguousarray(W_proj).astype(BF)                 # [C, C]
    bp = np.ascontiguousarray(b_proj.reshape(NKC, 128).T)        # [128, 8]

    cn = np.zeros((128, CN_W), dtype=BF)
    cn[:, CN_ID:CN_ID + 128] = np.eye(128, dtype=np.float32)
    tril = (np.arange(128)[None, :] >= np.arange(128)[:, None])
    cn[:, CN_MASK:CN_MASK + 128] = tril.astype(np.float32)
    cn[:, CN_MASK + 128:CN_MASK + 256] = tril.astype(np.float32)
    cn[:, CN_ONES:CN_ONES + 32] = 1.0
    cn[0, CN_ONESR:CN_ONESR + 64] = 1.0

    qw = W_qkv[:, 0:C]
    kw = W_qkv[:, C:2 * C]
    vw = W_qkv[:, 2 * C:3 * C]
    qb, kb, vb = b_qkv[0:C], b_qkv[C:2 * C], b_qkv[2 * C:3 * C]

    in_maps = []
    for c in range(NCORES):
        cols = slice(2 * c * HD, (2 * c + 2) * HD)  # this core's 128 dims
        bq = np.stack([qb[cols] * scale, kb[cols], vb[cols]], axis=1)  # [128,3]
        in_maps.append({
            "xt": xt,
            "wq": np.ascontiguousarray(qw[:, cols] * scale).astype(BF),
            "wk": np.ascontiguousarray(kw[:, cols]).astype(BF),
            "wv": np.ascontiguousarray(vw[:, cols]).astype(BF),
            "wp": wp,
            "cn": cn,
            "bqkv": np.ascontiguousarray(bq),
            "bp": bp,
        })

    nc = _get_nc()
    _CACHE["last_in_maps"] = in_maps
    res = run_bass_kernel_spmd(nc, in_maps, core_ids=list(range(NCORES)))

    # outp[c]: [C, B*PIECE] (cols: b-major, then 256 tokens of piece c)
    allo = np.stack([np.asarray(res.results[c]["outp"], dtype=np.float32)
                     for c in range(NCORES)])
    allo = allo.reshape(NCORES, C, B, PIECE)       # [c, ch, b, u]
    out = allo.transpose(2, 0, 3, 1).reshape(B, T, C)
    return np.ascontiguousarray(out)
